# revision 59
# baseline (speedup 1.0000x reference)
"""Causal self-attention (B=2, T=2048, D=1024, H=16, HD=64) on 8 trn2 NeuronCores.

Sharding: core = b*4 + g  (b = batch 0..1, g = head-group 0..3, 4 heads each).
Megatron-style: column-split w_qkv per head group, row-split w_o; the w_o
all-reduce over each batch's 4 cores is done host-side (partial sums).

Per-core device program (Tile framework, all-bf16 matmuls):
  1. qk projection evicted straight into a paired layout qk[128, s, T] bf16
     (s: 0,1 = q head pairs, 2,3 = k head pairs; local head h lives at
     partition rows 64*(h%2)..) so score matmuls contract K=64 at partition
     offset 0/64 with no staging copies.
  2. v[t, dv_ext] bf16; dv_ext = per head [64 v cols | 1.0 ones col] (ones col
     from zero weight col + bias 1.0) -> lhsT for the y matmul also yields the
     softmax denominator for free.
  3. Per (head, tq-chunk 512): scoresT[tk,tq] blocks, diagonal blocks width-
     restricted (512/384/256/128); exp on ACT with scale 1/32 (softmax
     max-subtraction provably unnecessary: scores/32 has std ~0.25); causal
     triangle + fill of the uncomputed region via one bf16 matmul against a
     [128,256] mask constant (PSUM has_written semantics overwrite the
     never-written region with -1e30); y^T accumulation in PSUM with denom
     row; reciprocal_approx_fast + partition broadcast + multiply normalizes.
  4. out_part[t, :] = yT.T @ w_o rows (bf16); host sums 4 partials per batch
     and adds b_o.
"""

import os
import numpy as np

B, T, D = 2, 2048, 1024
H, HD = 16, 64
LH = 4            # local heads per core
KO = 8            # contraction tiles of 128 over D
DVE_ = 260        # v columns per core incl. ones cols (4 * 65)
NTQ, TQC = 4, 512
NTKB, TKB = 16, 128

_PROG = {}
LAST_RESULT = None


def _build_program(debug_dumps=False):
    import concourse.bass as bass
    from concourse import bacc
    import concourse.tile as tile
    import concourse.mybir as mybir

    f32 = mybir.dt.float32
    bf16 = mybir.dt.bfloat16
    AF = mybir.ActivationFunctionType
    ts = bass.ts

    nc = bacc.Bacc(None, target_bir_lowering=False, debug=False)
    xT_d = nc.dram_tensor("xT", [128, NTQ, KO, TQC], bf16, kind="ExternalInput")
    wqk_d = nc.dram_tensor("w_qk", [128, KO, 512], bf16, kind="ExternalInput")
    bqk_d = nc.dram_tensor("b_qk", [128, 4], f32, kind="ExternalInput")
    wv_d = nc.dram_tensor("w_v", [128, KO, DVE_], bf16, kind="ExternalInput")
    bv_d = nc.dram_tensor("b_v", [128, DVE_], f32, kind="ExternalInput")
    wo_d = nc.dram_tensor("w_o", [128, 2, D], bf16, kind="ExternalInput")
    ident_d = nc.dram_tensor("ident", [128, 128], bf16, kind="ExternalInput")
    masks_d = nc.dram_tensor("masks", [128, 256], bf16, kind="ExternalInput")
    out_d = nc.dram_tensor("out_part", [T, D], bf16, kind="ExternalOutput")
    dbg = {}
    if debug_dumps:
        dbg["qk"] = nc.dram_tensor("dbg_qk", [128, 4, T], bf16, kind="ExternalOutput")
        dbg["v"] = nc.dram_tensor("dbg_v", [128, NTKB, LH, 65], bf16, kind="ExternalOutput")
        dbg["yT"] = nc.dram_tensor("dbg_yT", [128, 2, T], bf16, kind="ExternalOutput")

    with tile.TileContext(nc) as tc:
        with (
            tc.tile_pool(name="big", bufs=1) as big,
            tc.tile_pool(name="xtp", bufs=4) as xtp,
            tc.tile_pool(name="expp", bufs=4) as expp,
            tc.tile_pool(name="ev", bufs=3) as ev,
            tc.tile_pool(name="outp", bufs=3) as outp,
            tc.tile_pool(name="ps_misc", bufs=2, space="PSUM") as ps_misc,
            tc.tile_pool(name="ps_s", bufs=2, space="PSUM") as ps_sp,
            tc.tile_pool(name="ps_y", bufs=2, space="PSUM") as ps_yp,
        ):
            wqk = big.tile([128, KO, 512], bf16, name="wqk_sb")
            wv = big.tile([128, KO, DVE_], bf16, name="wv_sb")
            wo = big.tile([128, 2, D], bf16, name="wo_sb")
            bqk = big.tile([128, 4], f32, name="bqk_sb")
            bv = big.tile([128, LH, 65], f32, name="bv_sb")
            # paired layout: [128, s, T]; s 0,1 = q head pairs (0,1),(2,3),
            # s 2,3 = k head pairs; head h%2==0 at rows 0:64, h%2==1 at 64:128
            qk = big.tile([128, 4, T], bf16, name="qk_sb")
            vsb = big.tile([128, NTKB, LH, 65], bf16, name="v_sb")
            yT = big.tile([128, 2, T], bf16, name="yT_sb")
            ident = big.tile([128, 128], bf16, name="ident_sb")
            # msk[:, 0:128] = -1e30 everywhere; msk[:, 128+g] = -1e30 iff g<p
            msk = big.tile([128, 256], bf16, name="msk_sb")
            # single row of ones at partition 64 (stationary operand of the
            # denominator-broadcast matmul in the last chunk's normalize)
            ones64 = big.tile([128, 64], bf16, name="ones64_sb")
            nc.gpsimd.memset(ones64[64:65, :], 1.0)

            # startup DMA order = first-need order. Weights on the sync
            # queue, x chunks on the gpsimd queue so they stream in parallel.
            # Fine-grained first transfers so proj c0 streams at arrival rate.
            nc.sync.dma_start(wqk[:, 0, :], wqk_d[:, 0, :])
            nc.sync.dma_start(wqk[:, 1:4, :], wqk_d[:, 1:4, :])
            nc.sync.dma_start(wqk[:, 4:KO, :], wqk_d[:, 4:KO, :])
            # second-wave loads on the (idle until attention) scalar queue;
            # tiny constants first, bulk weights emitted after the x loads so
            # chunk 0's scalar-queue half isn't stuck behind them
            nc.scalar.dma_start(ident[:], ident_d[:])
            nc.scalar.dma_start(msk[:], masks_d[:])
            nc.scalar.dma_start(bqk[:], bqk_d[:])
            nc.scalar.dma_start(wv[:], wv_d[:])
            nc.scalar.dma_start(bv[:], bv_d[:].rearrange("p (h e) -> p h e", h=LH))
            nc.scalar.dma_start(wo[:], wo_d[:])

            def load_xchunk(c):
                xTc = xtp.tile([128, KO, TQC], bf16, name=f"xTc_{c}", tag="xTc")
                if c == 0:
                    nc.gpsimd.dma_start(xTc[:, 0, :], xT_d[:, 0, 0, :])
                    nc.gpsimd.dma_start(xTc[:, 1:4, :], xT_d[:, 0, 1:4, :])
                    nc.gpsimd.dma_start(xTc[:, 4:KO, :], xT_d[:, 0, 4:KO, :])
                else:
                    nc.gpsimd.dma_start(xTc[:], xT_d[:, c, :, :])
                return xTc

            def proj(c, xTc):
                # q,k projection for this chunk -> paired layout, no staging.
                # ko-major over an s-pair so the first chunk streams at DMA
                # arrival rate (each arriving ko slice feeds 2 matmuls).
                for sp in range(2):
                    psts = {}
                    for s in (2 * sp, 2 * sp + 1):
                        psts[s] = ps_misc.tile(
                            [128, 512], f32, name=f"ps1_{s}_{c}", tag="misc"
                        )
                    for ko in range(KO):
                        for s in (2 * sp, 2 * sp + 1):
                            nc.tensor.matmul(
                                psts[s][:],
                                wqk[:, ko, ts(s, 128)],
                                xTc[:, ko, :],
                                start=(ko == 0),
                                stop=(ko == KO - 1),
                            )
                    for s in (2 * sp, 2 * sp + 1):
                        nc.vector.tensor_scalar_add(
                            qk[:, s, ts(c, 512)], psts[s][:], bqk[:, s : s + 1]
                        )
                # v projection for tk blocks of this chunk (with ones column)
                for tbl in range(4):
                    tb = 4 * c + tbl
                    pst = ps_misc.tile([128, 512], f32, name=f"ps2_{tb}", tag="misc")
                    for ko in range(KO):
                        nc.tensor.matmul(
                            pst[:, 0:DVE_],
                            xTc[:, ko, ts(tbl, 128)],
                            wv[:, ko, :],
                            start=(ko == 0),
                            stop=(ko == KO - 1),
                        )
                    nc.vector.tensor_add(
                        vsb[:, tb, :, :],
                        pst[:, 0:DVE_].rearrange("p (h e) -> p h e", h=LH),
                        bv[:],
                    )

            def attention(c, hps=(0, 1)):
                nb = 4 * (c + 1)
                for hp in hps:
                    lhs = (2 * hp, 2 * hp + 1)
                    psys = {}
                    for lh in lhs:
                        psys[lh] = ps_yp.tile(
                            [128, 512], f32, name=f"psy_{lh}_{c}", tag="psy"
                        )
                    for j2 in range(nb // 2):
                        ets = {}
                        for lh in lhs:
                            off = 64 * (lh % 2)
                            sq, sk = lh // 2, 2 + lh // 2
                            pss = ps_sp.tile(
                                [128, 2, 512], f32, name=f"pss_{lh}_{c}_{j2}", tag="pss"
                            )
                            for j in range(2):
                                tb = 2 * j2 + j
                                jj = tb - (nb - 4)  # diag index 0..3, <0 = full
                                ofs = 128 * jj if jj > 0 else 0
                                # odd jj: also compute the 128 cols the mask
                                # matmul -1e30-fills, so the exp read region
                                # is real data regardless of PSUM history
                                sof = ofs - 128 if (jj >= 0 and jj % 2 == 1) else ofs
                                nc.tensor.matmul(
                                    pss[:, j, sof:512],
                                    qk[off : off + 64, sk, ts(tb, 128)],
                                    qk[off : off + 64, sq, 512 * c + sof : 512 * (c + 1)],
                                    start=True,
                                    stop=(jj < 0),
                                )
                                if jj >= 0:
                                    # triangle mask at [ofs, ofs+128) only;
                                    # for odd jj the cols [ofs-128, ofs) hold
                                    # real (unmasked) scores whose exp output
                                    # the width-restricted y matmul never
                                    # reads, so no -1e30 fill is needed
                                    nc.tensor.matmul(
                                        pss[:, j, ofs : ofs + 128],
                                        ident[:],
                                        msk[:, 128:256],
                                        start=False,
                                        stop=True,
                                    )
                            et = expp.tile(
                                [128, 2, 512], bf16, name=f"et_{lh}_{c}_{j2}", tag="et"
                            )
                            # restrict exp to the defined/needed region:
                            # second diag pair only covers cols 256:512
                            eo = 256 if (2 * j2) - (nb - 4) == 2 else 0
                            nc.scalar.activation(
                                et[:, :, eo:512], pss[:, :, eo:512], AF.Exp,
                                scale=1.0 / 32.0,
                            )
                            ets[lh] = et
                        for lh in lhs:
                            for j in range(2):
                                tb = 2 * j2 + j
                                jj = tb - (nb - 4)
                                ofs = 128 * jj if jj > 0 else 0
                                nc.tensor.matmul(
                                    psys[lh][0:65, ofs:512],
                                    vsb[:, tb, lh, :],
                                    ets[lh][:, j, ofs:512],
                                    start=(tb == 0),
                                    stop=(tb == nb - 1),
                                )
                    rbs, bcps = {}, {}
                    if c == 0:
                        # short pipelined chains for the exposed last chunk:
                        # both heads' bf16 denom rows + PE broadcasts first,
                        # then both reciprocals + muls
                        for lh in lhs:
                            rcb = ev.tile(
                                [128, 512], bf16, name=f"rcb_{lh}_{c}", tag="rcb"
                            )
                            nc.vector.tensor_copy(rcb[64:65, :], psys[lh][64:65, :])
                            bcp = ps_misc.tile(
                                [128, 512], f32, name=f"bc_{lh}_{c}", tag="misc"
                            )
                            nc.tensor.matmul(
                                bcp[0:64, :], ones64[64:65, :], rcb[64:65, :],
                                start=True, stop=True,
                            )
                            bcps[lh] = bcp
                    for lh in lhs:
                        psy = psys[lh]
                        kt_y = lh // 2
                        rb = ev.tile([128, 512], f32, name=f"rb_{lh}_{c}", tag="rb")
                        if c == 0:
                            nc.vector.reciprocal_approx_fast(
                                rb[0:64, :], bcps[lh][0:64, :]
                            )
                        else:
                            # denom row -> SBUF -> partition 0 -> broadcast,
                            # then approx-reciprocal on the broadcast block
                            # (the custom DVE op mishandles single-partition
                            # slices)
                            rc = ev.tile([128, 512], f32, name=f"rc_{lh}_{c}", tag="rc")
                            nc.vector.tensor_copy(rc[64:65, :], psy[64:65, :])
                            rc0 = ev.tile([1, 512], f32, name=f"rc0_{lh}_{c}", tag="rc0")
                            nc.sync.dma_start(rc0[:], rc[64:65, :])
                            db = ev.tile([128, 512], f32, name=f"db_{lh}_{c}", tag="db")
                            nc.gpsimd.partition_broadcast(db[:], rc0[:])
                            nc.vector.reciprocal_approx_fast(rb[0:64, :], db[0:64, :])
                        if lh % 2 == 0:
                            nc.vector.tensor_mul(
                                yT[0:64, kt_y, ts(c, 512)], psy[0:64, :], rb[0:64, :]
                            )
                        else:
                            tmp = ev.tile(
                                [64, 512], bf16, name=f"tmp_{lh}_{c}", tag="tmpy"
                            )
                            nc.vector.tensor_mul(tmp[:], psy[0:64, :], rb[0:64, :])
                            nc.sync.dma_start(yT[64:128, kt_y, ts(c, 512)], tmp[:])

            def oproj(c, split_kt=False, evict_act=False):
                cp = nc.scalar.copy if evict_act else nc.vector.tensor_copy
                # split_kt: contract kt=0 (heads 0,1) as K=64 matmuls per head
                # half so the output projection starts before the second head
                # pair's normalize completes (used for the last chunk)
                for mb in range(4):
                    m = 4 * c + mb
                    ot = outp.tile([128, D], bf16, name=f"ot_{m}", tag="ot")
                    for n in range(2):
                        pst = ps_misc.tile(
                            [128, 512], f32, name=f"ps4_{m}_{n}", tag="misc"
                        )
                        if split_kt:
                            # kt halves in separate accumulators (kt0 only
                            # needs head pair 0's normalize); evict kt0 to
                            # SBUF then add kt1's PSUM on DVE (a tensor op
                            # may only take one PSUM operand)
                            pst2 = ps_misc.tile(
                                [128, 512], f32, name=f"ps4b_{m}_{n}", tag="misc"
                            )
                            ots = ev.tile(
                                [128, 512], f32, name=f"ots_{m}_{n}", tag="ots"
                            )
                            nc.tensor.matmul(
                                pst[:], yT[:, 0, ts(m, 128)], wo[:, 0, ts(n, 512)],
                                start=True, stop=True,
                            )
                            cp(ots[:], pst[:])
                            nc.tensor.matmul(
                                pst2[:], yT[:, 1, ts(m, 128)], wo[:, 1, ts(n, 512)],
                                start=True, stop=True,
                            )
                            nc.vector.tensor_add(ot[:, ts(n, 512)], ots[:], pst2[:])
                        else:
                            for kt in range(2):
                                nc.tensor.matmul(
                                    pst[:],
                                    yT[:, kt, ts(m, 128)],
                                    wo[:, kt, ts(n, 512)],
                                    start=(kt == 0),
                                    stop=(kt == 1),
                                )
                            cp(ot[:, ts(n, 512)], pst[:])
                    nc.sync.dma_start(out_d[ts(m, 128), :], ot[:])

            # chunk order: proj 0,1,2,3 interleaved with attention 1,2,3,0.
            # The big ACT-bound attention chunks overlap proj/oproj PE work;
            # the tiny chunk 0 runs last so the endgame has filler (oproj 3,0)
            # and the PE never idles long enough to re-throttle the clock.
            # All x loads issued upfront so nothing queues behind attention's
            # gpsimd work.
            xs = [load_xchunk(c) for c in range(NTQ)]
            # ~3.4us of inert matmuls on the early-arriving constants: keeps
            # the PE busy while x/w stream in, so the HAM clock gate opens
            # (1.2 -> 2.4 GHz) before the real work starts instead of ~10us
            # into it
            warm = ps_misc.tile([128, 256], f32, name="warm", tag="misc")
            for i in range(28):
                nc.tensor.matmul(
                    warm[:], wqk[:, 0, 0:128], wqk[:, 0, 0:256],
                    start=(i == 0), stop=(i == 27),
                )
            proj(0, xs[0])
            proj(1, xs[1])
            attention(1)
            proj(2, xs[2])
            attention(2)
            proj(3, xs[3])
            oproj(1)
            attention(3, hps=(0,))
            oproj(2)
            attention(3, hps=(1,))
            attention(0, hps=(0,))
            oproj(3)
            attention(0, hps=(1,))
            oproj(0, split_kt=True)

            if debug_dumps:
                nc.sync.dma_start(dbg["qk"][:], qk[:])
                nc.sync.dma_start(dbg["v"][:], vsb[:])
                nc.sync.dma_start(dbg["yT"][:], yT[:])

    nc.finalize()
    return nc


def _mask_tile():
    # [128, 256] bf16: cols 0:128 = -1e30 (fill for uncomputed regions);
    # cols 128+g = -1e30 iff g < p (strict lower triangle)
    import ml_dtypes

    p = np.arange(128)[:, None]
    g = np.arange(128)[None, :]
    fill = np.full((128, 128), -1e30, dtype=np.float32)
    tri = np.where(g < p, -1e30, 0.0).astype(np.float32)
    return np.concatenate([fill, tri], axis=1).astype(ml_dtypes.bfloat16)


def kernel(x, w_qkv, b_qkv, w_o, b_o):
    global LAST_RESULT
    import ml_dtypes
    from concourse.bass_utils import run_bass_kernel_spmd

    bf16 = ml_dtypes.bfloat16
    x = np.asarray(x, dtype=np.float32)
    w_qkv = np.asarray(w_qkv, dtype=np.float32)
    b_qkv = np.asarray(b_qkv, dtype=np.float32)
    w_o = np.asarray(w_o, dtype=np.float32)
    b_o = np.asarray(b_o, dtype=np.float32)

    debug_dumps = bool(os.environ.get("KERNEL_DEBUG_DUMPS"))
    key = ("nc", debug_dumps)
    if key not in _PROG:
        _PROG[key] = _build_program(debug_dumps)
    nc = _PROG[key]

    # host-side shard prep
    xT = []
    for b in range(B):
        t = np.ascontiguousarray(x[b].T)  # [D, T]
        # [128, NTQ, KO, 512]: xT[p, c, ko, j] = x[b, 512c+j, 128ko+p]
        t4 = t.reshape(KO, 128, NTQ, TQC).transpose(1, 2, 0, 3)
        xT.append(np.ascontiguousarray(t4).astype(bf16))

    msk = _mask_tile()
    ident = np.eye(128, dtype=np.float32).astype(bf16)

    in_maps = []
    for core in range(8):
        b, g = divmod(core, 4)
        qcols = slice(g * 256, (g + 1) * 256)
        kcols = slice(D + g * 256, D + (g + 1) * 256)
        w_qk = np.concatenate([w_qkv[:, qcols], w_qkv[:, kcols]], axis=1)  # [D, 512]
        w_qk = np.ascontiguousarray(
            w_qk.reshape(KO, 128, 512).swapaxes(0, 1)
        ).astype(bf16)
        b_qk = np.concatenate([b_qkv[qcols], b_qkv[kcols]])  # [512]
        b_qk = np.ascontiguousarray(b_qk.reshape(4, 128).T)  # [128, 4]

        w_v = np.zeros((D, DVE_), dtype=np.float32)
        b_v = np.zeros((DVE_,), dtype=np.float32)
        for h in range(LH):
            vcols = slice(2 * D + g * 256 + h * 64, 2 * D + g * 256 + (h + 1) * 64)
            w_v[:, h * 65 : h * 65 + 64] = w_qkv[:, vcols]
            b_v[h * 65 : h * 65 + 64] = b_qkv[vcols]
            b_v[h * 65 + 64] = 1.0  # ones column (weight col stays 0)
        w_v = np.ascontiguousarray(w_v.reshape(KO, 128, DVE_).swapaxes(0, 1)).astype(
            bf16
        )
        b_v_bc = np.ascontiguousarray(np.tile(b_v[None, :], (128, 1)))

        w_o_g = w_o[g * 256 : (g + 1) * 256, :]  # [256, D]
        w_o_g = np.ascontiguousarray(w_o_g.reshape(2, 128, D).swapaxes(0, 1)).astype(
            bf16
        )

        in_maps.append(
            {
                "xT": xT[b],
                "w_qk": w_qk,
                "b_qk": np.ascontiguousarray(b_qk, dtype=np.float32),
                "w_v": w_v,
                "b_v": b_v_bc,
                "w_o": w_o_g,
                "ident": ident,
                "masks": msk,
            }
        )

    trace = bool(os.environ.get("KERNEL_TRACE"))
    res = run_bass_kernel_spmd(nc, in_maps, core_ids=list(range(8)), trace=trace)
    LAST_RESULT = res

    out = np.empty((B, T, D), dtype=np.float32)
    for b in range(B):
        acc = res.results[b * 4]["out_part"].astype(np.float32).copy()
        for g in range(1, 4):
            acc += res.results[b * 4 + g]["out_part"]
        out[b] = acc + b_o[None, :]
    return out


# revision 60
# speedup vs baseline: 1.0287x; 1.0287x over previous
"""Causal self-attention (B=2, T=2048, D=1024, H=16, HD=64) on 8 trn2 NeuronCores.

Sharding: core = b*4 + g  (b = batch 0..1, g = head-group 0..3, 4 heads each).
Megatron-style: column-split w_qkv per head group, row-split w_o; the w_o
all-reduce over each batch's 4 cores is done host-side (partial sums).

Per-core device program (Tile framework, all-bf16 matmuls):
  1. qk projection evicted straight into a paired layout qk[128, s, T] bf16
     (s: 0,1 = q head pairs, 2,3 = k head pairs; local head h lives at
     partition rows 64*(h%2)..) so score matmuls contract K=64 at partition
     offset 0/64 with no staging copies.
  2. v[t, dv_ext] bf16; dv_ext = per head [64 v cols | 1.0 ones col] (ones col
     from zero weight col + bias 1.0) -> lhsT for the y matmul also yields the
     softmax denominator for free.
  3. Per (head, tq-chunk 512): scoresT[tk,tq] blocks, diagonal blocks width-
     restricted (512/384/256/128); exp on ACT with scale 1/32 (softmax
     max-subtraction provably unnecessary: scores/32 has std ~0.25); causal
     triangle + fill of the uncomputed region via one bf16 matmul against a
     [128,256] mask constant (PSUM has_written semantics overwrite the
     never-written region with -1e30); y^T accumulation in PSUM with denom
     row; reciprocal_approx_fast + partition broadcast + multiply normalizes.
  4. out_part[t, :] = yT.T @ w_o rows (bf16); host sums 4 partials per batch
     and adds b_o.
"""

import os
import numpy as np

B, T, D = 2, 2048, 1024
H, HD = 16, 64
LH = 4            # local heads per core
KO = 8            # contraction tiles of 128 over D
DVE_ = 260        # v columns per core incl. ones cols (4 * 65)
NTQ, TQC = 4, 512
NTKB, TKB = 16, 128

_PROG = {}
LAST_RESULT = None


def _build_program(debug_dumps=False):
    import concourse.bass as bass
    from concourse import bacc
    import concourse.tile as tile
    import concourse.mybir as mybir

    f32 = mybir.dt.float32
    bf16 = mybir.dt.bfloat16
    AF = mybir.ActivationFunctionType
    ts = bass.ts

    nc = bacc.Bacc(None, target_bir_lowering=False, debug=False)
    xT_d = nc.dram_tensor("xT", [128, NTQ, KO, TQC], bf16, kind="ExternalInput")
    wqk_d = nc.dram_tensor("w_qk", [128, KO, 512], bf16, kind="ExternalInput")
    bqk_d = nc.dram_tensor("b_qk", [128, 4], f32, kind="ExternalInput")
    wv_d = nc.dram_tensor("w_v", [128, KO, DVE_], bf16, kind="ExternalInput")
    bv_d = nc.dram_tensor("b_v", [128, DVE_], f32, kind="ExternalInput")
    wo_d = nc.dram_tensor("w_o", [128, 2, D], bf16, kind="ExternalInput")
    ident_d = nc.dram_tensor("ident", [128, 128], bf16, kind="ExternalInput")
    masks_d = nc.dram_tensor("masks", [128, 256], bf16, kind="ExternalInput")
    out_d = nc.dram_tensor("out_part", [T, D], bf16, kind="ExternalOutput")
    dbg = {}
    if debug_dumps:
        dbg["qk"] = nc.dram_tensor("dbg_qk", [128, 4, T], bf16, kind="ExternalOutput")
        dbg["v"] = nc.dram_tensor("dbg_v", [128, NTKB, LH, 65], bf16, kind="ExternalOutput")
        dbg["yT"] = nc.dram_tensor("dbg_yT", [128, 2, T], bf16, kind="ExternalOutput")

    with tile.TileContext(nc) as tc:
        with (
            tc.tile_pool(name="big", bufs=1) as big,
            tc.tile_pool(name="xtp", bufs=4) as xtp,
            tc.tile_pool(name="expp", bufs=4) as expp,
            tc.tile_pool(name="ev", bufs=3) as ev,
            tc.tile_pool(name="outp", bufs=3) as outp,
            tc.tile_pool(name="ps_misc", bufs=2, space="PSUM") as ps_misc,
            tc.tile_pool(name="ps_s", bufs=2, space="PSUM") as ps_sp,
            tc.tile_pool(name="ps_y", bufs=2, space="PSUM") as ps_yp,
        ):
            wqk = big.tile([128, KO, 512], bf16, name="wqk_sb")
            wv = big.tile([128, KO, DVE_], bf16, name="wv_sb")
            wo = big.tile([128, 2, D], bf16, name="wo_sb")
            bqk = big.tile([128, 4], f32, name="bqk_sb")
            bv = big.tile([128, LH, 65], f32, name="bv_sb")
            # paired layout: [128, s, T]; s 0,1 = q head pairs (0,1),(2,3),
            # s 2,3 = k head pairs; head h%2==0 at rows 0:64, h%2==1 at 64:128
            qk = big.tile([128, 4, T], bf16, name="qk_sb")
            vsb = big.tile([128, NTKB, LH, 65], bf16, name="v_sb")
            yT = big.tile([128, 2, T], bf16, name="yT_sb")
            ident = big.tile([128, 128], bf16, name="ident_sb")
            # msk[:, 0:128] = -1e30 everywhere; msk[:, 128+g] = -1e30 iff g<p
            msk = big.tile([128, 256], bf16, name="msk_sb")
            # single row of ones at partition 64 (stationary operand of the
            # denominator-broadcast matmul in the last chunk's normalize)
            ones64 = big.tile([128, 64], bf16, name="ones64_sb")
            nc.gpsimd.memset(ones64[64:65, :], 1.0)

            # startup DMA order = first-need order. Weights on the sync
            # queue, x chunks on the gpsimd queue so they stream in parallel.
            # Fine-grained first transfers so proj c0 streams at arrival rate.
            nc.sync.dma_start(wqk[:, 0, :], wqk_d[:, 0, :])
            nc.sync.dma_start(wqk[:, 1:4, :], wqk_d[:, 1:4, :])
            nc.sync.dma_start(wqk[:, 4:KO, :], wqk_d[:, 4:KO, :])
            # second-wave loads on the (idle until attention) scalar queue;
            # tiny constants first, bulk weights emitted after the x loads so
            # chunk 0's scalar-queue half isn't stuck behind them
            nc.scalar.dma_start(ident[:], ident_d[:])
            nc.scalar.dma_start(msk[:], masks_d[:])
            nc.scalar.dma_start(bqk[:], bqk_d[:])
            nc.scalar.dma_start(wv[:], wv_d[:])
            nc.scalar.dma_start(bv[:], bv_d[:].rearrange("p (h e) -> p h e", h=LH))
            nc.scalar.dma_start(wo[:], wo_d[:])

            def load_xchunk(c):
                xTc = xtp.tile([128, KO, TQC], bf16, name=f"xTc_{c}", tag="xTc")
                if c == 0:
                    nc.gpsimd.dma_start(xTc[:, 0, :], xT_d[:, 0, 0, :])
                    nc.gpsimd.dma_start(xTc[:, 1:4, :], xT_d[:, 0, 1:4, :])
                    nc.gpsimd.dma_start(xTc[:, 4:KO, :], xT_d[:, 0, 4:KO, :])
                else:
                    nc.gpsimd.dma_start(xTc[:], xT_d[:, c, :, :])
                return xTc

            def proj(c, xTc):
                # q,k projection for this chunk -> paired layout, no staging.
                # ko-major over an s-pair so the first chunk streams at DMA
                # arrival rate (each arriving ko slice feeds 2 matmuls).
                for sp in range(2):
                    psts = {}
                    for s in (2 * sp, 2 * sp + 1):
                        psts[s] = ps_misc.tile(
                            [128, 512], f32, name=f"ps1_{s}_{c}", tag="misc"
                        )
                    for ko in range(KO):
                        for s in (2 * sp, 2 * sp + 1):
                            nc.tensor.matmul(
                                psts[s][:],
                                wqk[:, ko, ts(s, 128)],
                                xTc[:, ko, :],
                                start=(ko == 0),
                                stop=(ko == KO - 1),
                            )
                    for s in (2 * sp, 2 * sp + 1):
                        nc.vector.tensor_scalar_add(
                            qk[:, s, ts(c, 512)], psts[s][:], bqk[:, s : s + 1]
                        )
                # v projection for tk blocks of this chunk (with ones column)
                for tbl in range(4):
                    tb = 4 * c + tbl
                    pst = ps_misc.tile([128, 512], f32, name=f"ps2_{tb}", tag="misc")
                    for ko in range(KO):
                        nc.tensor.matmul(
                            pst[:, 0:DVE_],
                            xTc[:, ko, ts(tbl, 128)],
                            wv[:, ko, :],
                            start=(ko == 0),
                            stop=(ko == KO - 1),
                        )
                    nc.vector.tensor_add(
                        vsb[:, tb, :, :],
                        pst[:, 0:DVE_].rearrange("p (h e) -> p h e", h=LH),
                        bv[:],
                    )

            def attention(c, hps=(0, 1)):
                nb = 4 * (c + 1)
                for hp in hps:
                    lhs = (2 * hp, 2 * hp + 1)
                    psys = {}
                    for lh in lhs:
                        psys[lh] = ps_yp.tile(
                            [128, 512], f32, name=f"psy_{lh}_{c}", tag="psy"
                        )
                    for j2 in range(nb // 2):
                        ets = {}
                        for lh in lhs:
                            off = 64 * (lh % 2)
                            sq, sk = lh // 2, 2 + lh // 2
                            pss = ps_sp.tile(
                                [128, 2, 512], f32, name=f"pss_{lh}_{c}_{j2}", tag="pss"
                            )
                            for j in range(2):
                                tb = 2 * j2 + j
                                jj = tb - (nb - 4)  # diag index 0..3, <0 = full
                                ofs = 128 * jj if jj > 0 else 0
                                # odd jj: also compute the 128 cols the mask
                                # matmul -1e30-fills, so the exp read region
                                # is real data regardless of PSUM history
                                sof = ofs - 128 if (jj >= 0 and jj % 2 == 1) else ofs
                                nc.tensor.matmul(
                                    pss[:, j, sof:512],
                                    qk[off : off + 64, sk, ts(tb, 128)],
                                    qk[off : off + 64, sq, 512 * c + sof : 512 * (c + 1)],
                                    start=True,
                                    stop=(jj < 0),
                                )
                                if jj >= 0:
                                    # triangle mask at [ofs, ofs+128) only;
                                    # for odd jj the cols [ofs-128, ofs) hold
                                    # real (unmasked) scores whose exp output
                                    # the width-restricted y matmul never
                                    # reads, so no -1e30 fill is needed
                                    nc.tensor.matmul(
                                        pss[:, j, ofs : ofs + 128],
                                        ident[:],
                                        msk[:, 128:256],
                                        start=False,
                                        stop=True,
                                    )
                            et = expp.tile(
                                [128, 2, 512], bf16, name=f"et_{lh}_{c}_{j2}", tag="et"
                            )
                            # restrict exp to the defined/needed region:
                            # second diag pair only covers cols 256:512
                            eo = 256 if (2 * j2) - (nb - 4) == 2 else 0
                            nc.scalar.activation(
                                et[:, :, eo:512], pss[:, :, eo:512], AF.Exp,
                                scale=1.0 / 32.0,
                            )
                            ets[lh] = et
                        for lh in lhs:
                            for j in range(2):
                                tb = 2 * j2 + j
                                jj = tb - (nb - 4)
                                ofs = 128 * jj if jj > 0 else 0
                                nc.tensor.matmul(
                                    psys[lh][0:65, ofs:512],
                                    vsb[:, tb, lh, :],
                                    ets[lh][:, j, ofs:512],
                                    start=(tb == 0),
                                    stop=(tb == nb - 1),
                                )
                    rbs, bcps = {}, {}
                    if c == 0:
                        # short pipelined chains for the exposed last chunk:
                        # both heads' bf16 denom rows + PE broadcasts first,
                        # then both reciprocals + muls
                        for lh in lhs:
                            rcb = ev.tile(
                                [128, 512], bf16, name=f"rcb_{lh}_{c}", tag="rcb"
                            )
                            nc.vector.tensor_copy(rcb[64:65, :], psys[lh][64:65, :])
                            bcp = ps_misc.tile(
                                [128, 512], f32, name=f"bc_{lh}_{c}", tag="misc"
                            )
                            nc.tensor.matmul(
                                bcp[0:64, :], ones64[64:65, :], rcb[64:65, :],
                                start=True, stop=True,
                            )
                            bcps[lh] = bcp
                    for lh in lhs:
                        psy = psys[lh]
                        kt_y = lh // 2
                        rb = ev.tile([128, 512], f32, name=f"rb_{lh}_{c}", tag="rb")
                        if c == 0:
                            nc.vector.reciprocal_approx_fast(
                                rb[0:64, :], bcps[lh][0:64, :]
                            )
                        else:
                            # denom row -> SBUF -> partition 0 -> broadcast,
                            # then approx-reciprocal on the broadcast block
                            # (the custom DVE op mishandles single-partition
                            # slices)
                            rc = ev.tile([128, 512], f32, name=f"rc_{lh}_{c}", tag="rc")
                            nc.vector.tensor_copy(rc[64:65, :], psy[64:65, :])
                            rc0 = ev.tile([1, 512], f32, name=f"rc0_{lh}_{c}", tag="rc0")
                            nc.sync.dma_start(rc0[:], rc[64:65, :])
                            db = ev.tile([128, 512], f32, name=f"db_{lh}_{c}", tag="db")
                            nc.gpsimd.partition_broadcast(db[:], rc0[:])
                            nc.vector.reciprocal_approx_fast(rb[0:64, :], db[0:64, :])
                        if lh % 2 == 0:
                            nc.vector.tensor_mul(
                                yT[0:64, kt_y, ts(c, 512)], psy[0:64, :], rb[0:64, :]
                            )
                        else:
                            tmp = ev.tile(
                                [64, 512], bf16, name=f"tmp_{lh}_{c}", tag="tmpy"
                            )
                            nc.vector.tensor_mul(tmp[:], psy[0:64, :], rb[0:64, :])
                            nc.sync.dma_start(yT[64:128, kt_y, ts(c, 512)], tmp[:])

            def oproj(c, split_kt=False, evict_act=False):
                cp = nc.scalar.copy if evict_act else nc.vector.tensor_copy
                # split_kt: contract kt=0 (heads 0,1) as K=64 matmuls per head
                # half so the output projection starts before the second head
                # pair's normalize completes (used for the last chunk)
                for mb in range(4):
                    m = 4 * c + mb
                    ot = outp.tile([128, D], bf16, name=f"ot_{m}", tag="ot")
                    for n in range(2):
                        pst = ps_misc.tile(
                            [128, 512], f32, name=f"ps4_{m}_{n}", tag="misc"
                        )
                        if split_kt:
                            # kt halves in separate accumulators (kt0 only
                            # needs head pair 0's normalize); evict kt0 to
                            # SBUF then add kt1's PSUM on DVE (a tensor op
                            # may only take one PSUM operand)
                            pst2 = ps_misc.tile(
                                [128, 512], f32, name=f"ps4b_{m}_{n}", tag="misc"
                            )
                            ots = ev.tile(
                                [128, 512], f32, name=f"ots_{m}_{n}", tag="ots"
                            )
                            nc.tensor.matmul(
                                pst[:], yT[:, 0, ts(m, 128)], wo[:, 0, ts(n, 512)],
                                start=True, stop=True,
                            )
                            cp(ots[:], pst[:])
                            nc.tensor.matmul(
                                pst2[:], yT[:, 1, ts(m, 128)], wo[:, 1, ts(n, 512)],
                                start=True, stop=True,
                            )
                            nc.vector.tensor_add(ot[:, ts(n, 512)], ots[:], pst2[:])
                        else:
                            for kt in range(2):
                                nc.tensor.matmul(
                                    pst[:],
                                    yT[:, kt, ts(m, 128)],
                                    wo[:, kt, ts(n, 512)],
                                    start=(kt == 0),
                                    stop=(kt == 1),
                                )
                            cp(ot[:, ts(n, 512)], pst[:])
                    nc.sync.dma_start(out_d[ts(m, 128), :], ot[:])

            # chunk order: proj 0,1,2,3 interleaved with attention 1,2,3,0.
            # The big ACT-bound attention chunks overlap proj/oproj PE work;
            # the tiny chunk 0 runs last so the endgame has filler (oproj 3,0)
            # and the PE never idles long enough to re-throttle the clock.
            # All x loads issued upfront so nothing queues behind attention's
            # gpsimd work.
            xs = [load_xchunk(c) for c in range(NTQ)]
            # ~3.4us of inert matmuls on the early-arriving constants: keeps
            # the PE busy while x/w stream in, so the HAM clock gate opens
            # (1.2 -> 2.4 GHz) before the real work starts instead of ~10us
            # into it
            warm = ps_misc.tile([128, 256], f32, name="warm", tag="misc")
            for i in range(28):
                nc.tensor.matmul(
                    warm[:], wqk[:, 0, 0:128], wqk[:, 0, 0:256],
                    start=(i == 0), stop=(i == 27),
                )
            proj(0, xs[0])
            proj(1, xs[1])
            attention(1)
            proj(2, xs[2])
            attention(2)
            proj(3, xs[3])
            attention(3, hps=(0,))
            oproj(1)
            oproj(2)
            attention(3, hps=(1,))
            attention(0, hps=(0,))
            oproj(3)
            attention(0, hps=(1,))
            oproj(0, split_kt=True)

            if debug_dumps:
                nc.sync.dma_start(dbg["qk"][:], qk[:])
                nc.sync.dma_start(dbg["v"][:], vsb[:])
                nc.sync.dma_start(dbg["yT"][:], yT[:])

    nc.finalize()
    return nc


def _mask_tile():
    # [128, 256] bf16: cols 0:128 = -1e30 (fill for uncomputed regions);
    # cols 128+g = -1e30 iff g < p (strict lower triangle)
    import ml_dtypes

    p = np.arange(128)[:, None]
    g = np.arange(128)[None, :]
    fill = np.full((128, 128), -1e30, dtype=np.float32)
    tri = np.where(g < p, -1e30, 0.0).astype(np.float32)
    return np.concatenate([fill, tri], axis=1).astype(ml_dtypes.bfloat16)


def kernel(x, w_qkv, b_qkv, w_o, b_o):
    global LAST_RESULT
    import ml_dtypes
    from concourse.bass_utils import run_bass_kernel_spmd

    bf16 = ml_dtypes.bfloat16
    x = np.asarray(x, dtype=np.float32)
    w_qkv = np.asarray(w_qkv, dtype=np.float32)
    b_qkv = np.asarray(b_qkv, dtype=np.float32)
    w_o = np.asarray(w_o, dtype=np.float32)
    b_o = np.asarray(b_o, dtype=np.float32)

    debug_dumps = bool(os.environ.get("KERNEL_DEBUG_DUMPS"))
    key = ("nc", debug_dumps)
    if key not in _PROG:
        _PROG[key] = _build_program(debug_dumps)
    nc = _PROG[key]

    # host-side shard prep
    xT = []
    for b in range(B):
        t = np.ascontiguousarray(x[b].T)  # [D, T]
        # [128, NTQ, KO, 512]: xT[p, c, ko, j] = x[b, 512c+j, 128ko+p]
        t4 = t.reshape(KO, 128, NTQ, TQC).transpose(1, 2, 0, 3)
        xT.append(np.ascontiguousarray(t4).astype(bf16))

    msk = _mask_tile()
    ident = np.eye(128, dtype=np.float32).astype(bf16)

    in_maps = []
    for core in range(8):
        b, g = divmod(core, 4)
        qcols = slice(g * 256, (g + 1) * 256)
        kcols = slice(D + g * 256, D + (g + 1) * 256)
        w_qk = np.concatenate([w_qkv[:, qcols], w_qkv[:, kcols]], axis=1)  # [D, 512]
        w_qk = np.ascontiguousarray(
            w_qk.reshape(KO, 128, 512).swapaxes(0, 1)
        ).astype(bf16)
        b_qk = np.concatenate([b_qkv[qcols], b_qkv[kcols]])  # [512]
        b_qk = np.ascontiguousarray(b_qk.reshape(4, 128).T)  # [128, 4]

        w_v = np.zeros((D, DVE_), dtype=np.float32)
        b_v = np.zeros((DVE_,), dtype=np.float32)
        for h in range(LH):
            vcols = slice(2 * D + g * 256 + h * 64, 2 * D + g * 256 + (h + 1) * 64)
            w_v[:, h * 65 : h * 65 + 64] = w_qkv[:, vcols]
            b_v[h * 65 : h * 65 + 64] = b_qkv[vcols]
            b_v[h * 65 + 64] = 1.0  # ones column (weight col stays 0)
        w_v = np.ascontiguousarray(w_v.reshape(KO, 128, DVE_).swapaxes(0, 1)).astype(
            bf16
        )
        b_v_bc = np.ascontiguousarray(np.tile(b_v[None, :], (128, 1)))

        w_o_g = w_o[g * 256 : (g + 1) * 256, :]  # [256, D]
        w_o_g = np.ascontiguousarray(w_o_g.reshape(2, 128, D).swapaxes(0, 1)).astype(
            bf16
        )

        in_maps.append(
            {
                "xT": xT[b],
                "w_qk": w_qk,
                "b_qk": np.ascontiguousarray(b_qk, dtype=np.float32),
                "w_v": w_v,
                "b_v": b_v_bc,
                "w_o": w_o_g,
                "ident": ident,
                "masks": msk,
            }
        )

    trace = bool(os.environ.get("KERNEL_TRACE"))
    res = run_bass_kernel_spmd(nc, in_maps, core_ids=list(range(8)), trace=trace)
    LAST_RESULT = res

    out = np.empty((B, T, D), dtype=np.float32)
    for b in range(B):
        acc = res.results[b * 4]["out_part"].astype(np.float32).copy()
        for g in range(1, 4):
            acc += res.results[b * 4 + g]["out_part"]
        out[b] = acc + b_o[None, :]
    return out
